# revision 53
# baseline (speedup 1.0000x reference)
"""Trainium2 Bass kernel for a dense transformer attention block (nn_AttnBlock).

Reference computation (per batch b, C=256 channels, S=64*64=4096 positions):
  xt = x[b].reshape(C, S).T; xn = LN(xt)
  per head h (4 heads, d=64): q/k/v = xn_h @ w{q,k,v} + b{q,k,v}
  attn = softmax(q k^T / 8); o = attn @ v
  ao = concat_heads(o) @ wo + bo; av = ao + xt
  out = gelu(LN(av) @ w1 + b1) @ w2 + b2 + av

Sharding: 8 cores = 4 batches x 2 sequence halves. Each core gets its batch's
full x with columns rotated so its q-half is always columns 0..2047 (attention
is permutation-invariant over key positions, so the rotation is transparent
and keeps the program SPMD-identical). k/v are computed for the full sequence
(duplicated across the 2 cores of a batch - negligible), q and everything
after attention only for the core's half. No collectives.

Layout: channel-major [c, s] everywhere (the native input layout), so no
transposes are ever needed: projections/FFN contract over channels, which sit
on partitions. LayerNorm partition-dim sums use PE ones-matmuls with M=128 so
sums arrive replicated across partitions (broadcast for free). Attention uses
transposed scores [k, q]; exp needs no max-subtraction (LN'd inputs and the
1/sqrt(d) scale keep scores O(1), far from fp32 exp overflow). The softmax
denominator falls out of the attn@v matmul via an augmented ones column in v.
"""

import os
import sys

if "/opt/trn_rl_repo" not in sys.path:
    sys.path.insert(0, "/opt/trn_rl_repo")

import numpy as np

import concourse.bass as bass
import concourse.bacc as bacc
import concourse.mybir as mybir
from concourse import bass_utils
from concourse import tile as tile_mod
from concourse.tile import TileContext
from concourse.vector_clock import ScopedClock, VectorClock

F32 = mybir.dt.float32
F32R = mybir.dt.float32r
FP8 = mybir.dt.float8e4
I8 = mybir.dt.int8
U8 = mybir.dt.uint8
AF = mybir.ActivationFunctionType
OP = mybir.AluOpType

EMB, HEADS, HD = 256, 4, 64
BS, SZ = 4, 64
SEQ = SZ * SZ          # 4096
SH = SEQ // 2          # 2048 (per-core q half)
EPS = 1e-5
CK = 512               # chunk width for LN / projections
NKT = SEQ // 128       # 32 k-tiles
NP = NKT // 2          # 16 k-tile pairs (fp8 DoubleRow contracts 2 at once)
VB = 96                # v8 block: 64 v cols + ones col (64) + 31 pad
                       # (DoubleRow out partitions must be a multiple of 32)
# Attention-weight fp8 encoding: scores arrive from PE pre-scaled by
# SSCALE = 8/ln2, so e4m3 BITS of the weight are just trunc(sc + B0):
# weight = 2^((bits-56)/8) = exp(score) * 2^((B0-56)/8) -- the uniform
# 2^((B0-56)/8) factor cancels in the softmax normalization. B0 keeps the
# bits inside [0, 119] (no fp8 NaN) for |score| <= 7.3 sigma.
B0 = 34.0
SSCALE = 11.541560327111707     # 8 / ln(2)
ACT_EXP_BIAS = (B0 - 56.0) * 0.08664339756999316  # (B0-56)*ln2/8
# per-chunk engine split of the 16 exp pairs: A=ACT (true exp), D=DVE
# (bit-trick); ~9/7 balances ACT vs DVE+norm work. Strictly alternating at
# the chunk tail so the final ot drain is not gated by a serial ACT cluster.
EXP_PAT = "ADAADADADADADADA"
# Debug bisection knobs; hardcoded full for the shipped kernel so stray
# environment variables can never truncate the computation.
PHASE = 4
SUB = 9

# Matmul dtype knobs (float32r = 1 cycle/row for N>=256, float32 = 4 cycles)
BF16 = mybir.dt.bfloat16
SCORES_DT = BF16  # kT/qT stored bf16
PROJ_DT = F32R   # wo, w1, w2
STATS_DT = F32R  # LN partition sums (f32r: ~1e-4 rel, 4x faster)


def _patch_tile_drain():
    """Split the end-of-kernel drain's sem waits across SP nops: the CoreV3
    TPB_CTRL encoding supports fewer sync-wait slots than the global clock
    needs, so a single Drain carrying every proc's wait fails codegen."""
    if getattr(tile_mod.TileContext, "_drain_patched", False):
        return

    def _drain_and_barrier(self, tick_clock, wait_clock):
        for proc, tick in enumerate(list(tick_clock.global_clock)):
            if tick == 0:
                continue
            c = VectorClock()
            c.require_at_least(proc, tick)
            nop = self.nc.sync.nop(nofuse=True, hint=f"drain_wait_p{proc}")
            wait_clock.add_sem_waits(nop.ins, ScopedClock({None: c}))
        self.nc.sync.drain()
        self.nc.all_engine_barrier()
        assert self.sems is not None
        popped = self.nc._tile_sem_poison_stack.pop()
        assert popped is self._sem_poison
        self.nc.clear_and_free_semaphores(list(self.sems.allocated().values()))
        self.nc.all_engine_barrier()

    tile_mod.TileContext._drain_and_barrier = _drain_and_barrier
    tile_mod.TileContext._drain_patched = True


def _patch_act_tables():
    """The act-table-set picker chooses per-function greedily and ping-pongs
    between `exp_and_others` (Square/Exp) and `natural_log` (Ln) every LN
    chunk -- ~2.7us per reload. All functions this kernel uses live together
    in `natural_log_exp_and_others` (+ Gelu in `gelu_and_others`), so empty
    every other set (indices preserved) to force a single resident set."""
    import concourse.hw_specs as hw_specs

    if getattr(hw_specs, "_act_tables_patched", False):
        return
    _orig = hw_specs.get_activation_tables
    allowed = {"natural_log_exp_and_others", "gelu_and_others"}

    def _gat(arch):
        tabs = _orig(arch)
        return {k: (v if k in allowed else set()) for k, v in tabs.items()}

    hw_specs.get_activation_tables = _gat
    hw_specs._act_tables_patched = True
    import concourse.bacc as bacc_mod

    bacc_mod.get_activation_tables = _gat
    try:
        import concourse.bass_interp as bi

        bi.get_activation_tables = _gat
    except Exception:
        pass


def _patch_sbuf_limit():
    # tile_utils caps pool SBUF at 192KB/partition; cayman usable is ~208KB.
    try:
        from concourse import tile_utils

        if getattr(tile_utils, "max_sbuf_usage", 0) < 206 * 1024:
            tile_utils.max_sbuf_usage = 206 * 1024
    except Exception:
        pass


def _mm(nc, out, lhsT, rhs, dt, **kw):
    nc.tensor.matmul(out, lhsT.bitcast(dt), rhs.bitcast(dt), **kw)


def build(debug=False):
    _patch_tile_drain()
    _patch_sbuf_limit()
    _patch_act_tables()
    nc = bacc.Bacc(trn_type="TRN2")

    x_d = nc.dram_tensor("x", [EMB, SEQ], F32, kind="ExternalInput")
    # packed constants (built host-side in make_in_maps) so startup issues
    # 3 DMAs instead of ~35 serial HWDGE issues (~0.63us each):
    # wqkv: 6 block-diag [128,128] mats (wq/wk/wv x t-half), ln1_g folded in
    #       (and SSCALE/8 folded into wq)
    # wpk:  [wo_hi | wo_lo | w1g_hi | w1g_lo | w2_hi | w2_lo] -> [128, 1536]
    # vecs: [bo_tot(2) | b1_tot(2) | b2(2) | bq_tot_t0 | bq_tot_t1]
    wqkv_d = nc.dram_tensor("wqkv", [128, 6 * 128], F32,
                            kind="ExternalInput")
    wpk_d = nc.dram_tensor("wpk", [128, 6 * EMB], F32, kind="ExternalInput")
    vecs_d = nc.dram_tensor("vecs", [128, 8], F32, kind="ExternalInput")
    out_d = nc.dram_tensor("out", [EMB, SH], F32, kind="ExternalOutput")
    dbg = {}
    if debug:
        for name, shape in [("oall", [EMB, SH]), ("av", [EMB, SH])]:
            dbg[name] = nc.dram_tensor("dbg_" + name, shape, F32,
                                       kind="ExternalOutput")

    with TileContext(nc) as tc:
        with (
            tc.tile_pool(name="const", bufs=1) as cpool,
            tc.tile_pool(name="main", bufs=1) as mpool,
        ):
            # ---- constants (3 packed DMAs; see dram decls) --------------
            wqkv_sb = cpool.tile([128, 6 * 128], F32, name="wqkv_sb",
                                 tag="wqkv_sb")
            nc.gpsimd.dma_start(wqkv_sb[:], wqkv_d.ap()[:])
            vecs_sb = cpool.tile([128, 8], F32, name="vecs_sb", tag="vecs_sb")
            nc.gpsimd.dma_start(vecs_sb[:], vecs_d.ap()[:])
            wpk_sb = cpool.tile([128, 6 * EMB], F32, name="wpk_sb",
                                tag="wpk_sb")
            nc.gpsimd.dma_start(wpk_sb[:], wpk_d.ap()[:])
            # ones value 1/EMB: the LN stats matmuls directly produce the
            # mean and E[x^2] (replicated across partitions).
            ones = cpool.tile([128, 128], F32R, name="ones", tag="ones")
            nc.vector.memset(ones[:].bitcast(mybir.dt.uint32), 0x3B800000)
            # bf16 copies of the block-diag projection weights [w][t]
            w_bd = [[cpool.tile([128, 128], BF16, name=f"wbd{w}{t}",
                                tag=f"wbd{w}{t}") for t in range(2)]
                    for w in range(3)]
            for w in range(3):
                for t in range(2):
                    nc.vector.tensor_copy(
                        w_bd[w][t][:],
                        wqkv_sb[:, (w * 2 + t) * 128:(w * 2 + t + 1) * 128])
            wo_sb = [cpool.tile([128, EMB], F32R, name=f"wo{i}", tag=f"wo{i}") for i in range(2)]
            w1_sb = [cpool.tile([128, EMB], F32R, name=f"w1{i}", tag=f"w1{i}") for i in range(2)]
            w2_sb = [cpool.tile([128, EMB], F32R, name=f"w2{i}", tag=f"w2{i}") for i in range(2)]
            for i in range(2):
                for j, wt in enumerate([wo_sb[i], w1_sb[i], w2_sb[i]]):
                    nc.vector.tensor_copy(
                        wt[:], wpk_sb[:, (2 * j + i) * EMB:(2 * j + i + 1) * EMB])
            vsb = {"bo_tot": vecs_sb[:, 0:2], "b1_tot": vecs_sb[:, 2:4],
                   "b2": vecs_sb[:, 4:6]}
            bq_t = vecs_sb[:, 6:8]
            epsv = cpool.tile([128, 1], F32, name="epsv", tag="epsv")
            nc.vector.memset(epsv[:], EPS)
            expb = cpool.tile([128, 1], F32, name="expb", tag="expb")
            nc.vector.memset(expb[:], float(ACT_EXP_BIAS))

            # ---- persistent activations ---------------------------------
            # v8: fp8 DoubleRow weights layout [hh][pair][tile j][VB cols]
            # where cols 0..63 = v, col 64 = ones (denominator), 65.. = pad.
            x_q = [mpool.tile([128, SH], F32, name=f"xq{t}", tag=f"xq{t}") for t in range(2)]
            qT = [mpool.tile([128, SH], BF16, name=f"qT{t}", tag=f"qT{t}") for t in range(2)]
            kT = [mpool.tile([128, SEQ], BF16, name=f"kT{t}", tag=f"kT{t}") for t in range(2)]
            v8 = [mpool.tile([128, 2 * NP * 2 * VB], FP8, name=f"v8{t}", tag=f"v8{t}") for t in range(2)]

            # ones columns of v8 (softmax denominator rows); fp8 1.0 = 0x38
            for t in range(2):
                nc.vector.memset(
                    v8[t][:].bitcast(U8).rearrange(
                        "p (n e) -> p n e", e=VB)[:, :, HD:HD + 1],
                    0x38)

            def emit_ln_stats(lw, lps, xa, xb, x2_act=True, ab_bufs=8,
                              s_dt=F32):
                """LN-over-channels stats for one [128, CK] c-major pair
                (xa = c0..127, xb = c128..255).  S' = mean, Q' = E[x^2] via
                1/EMB-ones matmuls; var = Q'-S'^2; A = rstd =
                exp(-0.5 ln(var+eps)).  Returns bf16 SBUF (Scp, A); the psum
                ring frees as soon as SS/Scp/Vp are done, so stats of later
                chunks can run far ahead of their consumers.  ln gamma/beta
                are folded into downstream projection weights host-side."""
                x2a = lw.tile([128, CK], F32R, name="x2a", tag="x2a", bufs=3)
                x2b = lw.tile([128, CK], F32R, name="x2b", tag="x2b", bufs=3)
                if x2_act:
                    nc.scalar.activation(x2a[:], xa.bitcast(F32), AF.Square)
                else:
                    nc.vector.tensor_mul(x2a[:], xa.bitcast(F32),
                                         xa.bitcast(F32))
                nc.gpsimd.tensor_mul(x2b[:], xb.bitcast(F32),
                                     xb.bitcast(F32))
                S = lps.tile([128, CK], F32, name="S", tag="S", bufs=2)
                Q = lps.tile([128, CK], F32, name="Q", tag="Q", bufs=2)
                # s_dt is F32 for raw-DMA inputs (walrus rejects f32r
                # matmuls whose input wasn't produced rounded) and F32R for
                # compute-produced inputs like av.
                _mm(nc, S[:], ones[:].bitcast(s_dt), xa, s_dt,
                    start=True, stop=False)
                _mm(nc, S[:], ones[:].bitcast(s_dt), xb, s_dt,
                    start=False, stop=True)
                _mm(nc, Q[:], ones[:], x2a[:], STATS_DT, start=True, stop=False)
                _mm(nc, Q[:], ones[:], x2b[:], STATS_DT, start=False, stop=True)
                SS = lw.tile([128, CK], F32, name="SS", tag="SS", bufs=3)
                nc.scalar.activation(SS[:], S[:], AF.Square)
                Scp = lw.tile([128, CK], BF16, name="Scp", tag="Scp",
                              bufs=ab_bufs)
                nc.scalar.copy(Scp[:], S[:])
                Vp = lw.tile([128, CK], F32, name="Vp", tag="Vp", bufs=3)
                nc.vector.tensor_sub(Vp[:], Q[:], SS[:])
                L = lw.tile([128, CK], F32, name="L", tag="L", bufs=3)
                nc.scalar.activation(L[:], Vp[:], AF.Ln, bias=epsv[:, 0:1])
                A = lw.tile([128, CK], BF16, name="A", tag="A", bufs=ab_bufs)
                nc.scalar.activation(A[:], L[:], AF.Exp, scale=-0.5)
                return Scp, A

            def emit_ln_apply(lw, xa, xb, Scp, A, xn0, xn1):
                for xi, xno, half in ((xa, xn0, 0), (xb, xn1, 1)):
                    u = lw.tile([128, CK], F32, name=f"u{half}",
                                tag=f"u{half}", bufs=3)
                    nc.vector.tensor_sub(u[:], xi.bitcast(F32), Scp[:])
                    nc.gpsimd.tensor_mul(xno, u[:], A[:])

            # ================= LN1 + q/k/v projections ===================
            # two passes: a stats pass whose PE matmuls only carry constant
            # stationary weights (so PE never stalls waiting on the LN
            # chain), and an apply+projection pass trailing it. Scp/A SBUF
            # rings decouple the passes; the S psum ring is freed early.
            with (
                tc.tile_pool(name="lnw", bufs=2) as lw,
                tc.tile_pool(name="ln_ps", bufs=2, space="PSUM") as lps,
                tc.tile_pool(name="pj_ps", bufs=2, space="PSUM") as pps,
            ):
                NCH = SEQ // CK
                xpairs = []
                stats = []
                for ch in range(NCH):
                    qhalf = ch < SH // CK
                    if ch % 2 == 0:
                        # one [128, 2*CK] load per tile-row per chunk-pair
                        psl = slice(ch * CK, (ch + 2) * CK)
                        xp = (lw.tile([128, 2 * CK], F32, name="xp0",
                                      tag="xp0", bufs=4),
                              lw.tile([128, 2 * CK], F32, name="xp1",
                                      tag="xp1", bufs=4))
                        nc.sync.dma_start(xp[0][:], x_d.ap()[0:128, psl])
                        nc.sync.dma_start(xp[1][:], x_d.ap()[128:256, psl])
                        xpairs.append(xp)
                    co = (ch % 2) * CK
                    xa = xpairs[ch // 2][0][:, co:co + CK]
                    xb = xpairs[ch // 2][1][:, co:co + CK]
                    Scp, A = emit_ln_stats(lw, lps, xa, xb, x2_act=not qhalf,
                                           ab_bufs=NCH)
                    stats.append((xa, xb, Scp, A))
                for ch in range(NCH):
                    sl = slice(ch * CK, (ch + 1) * CK)
                    qhalf = ch < SH // CK
                    xa, xb, Scp, A = stats[ch]
                    xn0 = lw.tile([128, CK], BF16, name="xn0", tag="xn0",
                                  bufs=3)
                    xn1 = lw.tile([128, CK], BF16, name="xn1", tag="xn1",
                                  bufs=3)
                    emit_ln_apply(lw, xa, xb, Scp, A, xn0[:], xn1[:])
                    for t, xn in ((0, xn0), (1, xn1)):
                        # k/q projections: block-diag weights give both heads
                        # in one matmul, partitions already head-major.
                        kps = pps.tile([128, CK], F32, name="kps",
                                       tag="kq", bufs=3)
                        _mm(nc, kps[:], w_bd[1][t][:], xn[:], BF16,
                            start=True, stop=True)
                        # k copies: ACT except one DVE on q-half chunks
                        if qhalf and t == 1:
                            nc.vector.tensor_copy(kT[t][:, sl], kps[:])
                        else:
                            nc.scalar.copy(kT[t][:, sl], kps[:])
                        if qhalf:
                            qps = pps.tile([128, CK], F32, name="qps",
                                           tag="kq", bufs=3)
                            _mm(nc, qps[:], w_bd[0][t][:], xn[:], BF16,
                                start=True, stop=True)
                            if t == 0:
                                nc.scalar.activation(qT[t][:, sl], qps[:],
                                                     AF.Identity,
                                                     bias=bq_t[:, t:t + 1])
                            else:
                                nc.vector.tensor_scalar_add(
                                    qT[t][:, sl], qps[:], bq_t[:, t:t + 1])
                        # v for the 4 s-tiles: one psum bank, one fp8 copy
                        vps = pps.tile([128, CK], F32, name="vps",
                                       tag="vps", bufs=1)
                        for st in range(CK // 128):
                            _mm(nc, vps[:, st * 128:(st + 1) * 128],
                                xn[:, st * 128:(st + 1) * 128],
                                w_bd[2][t][:], BF16, start=True, stop=True)
                        # dest: [hh][pair 2ch..2ch+1][tile j][0:64]
                        p0 = 2 * ch * VB * 2
                        dst = v8[t][:].rearrange("p (h r) -> p h r", h=2)[
                            :, :, p0:p0 + 4 * VB].rearrange(
                            "p h (s e) -> p h s e", e=VB)[:, :, :, 0:HD]
                        src = vps[:].rearrange("p (s h e) -> p h s e",
                                               s=4, h=2)
                        nc.vector.tensor_copy(dst, src)

            # residual copy of the q half: straight from DRAM (consumed
            # only by the wo stage, ~150us later)
            for t in range(2):
                nc.sync.dma_start(x_q[t][:], x_d.ap()[t * 128:(t + 1) * 128,
                                                      0:SH])

            if PHASE == 1:
                for t in range(2):
                    nc.sync.dma_start(
                        out_d.ap()[t * 128:(t + 1) * 128, :], x_q[t][:])

            # pools (LIFO): post (av, xn2) > opool (o_all) > attention work
            with tc.tile_pool(name="post", bufs=1) as pp:
                av = [pp.tile([128, SH], F32R, name=f"av{t}", tag=f"av{t}") for t in range(2)]
                xn2 = [pp.tile([128, SH], F32R, name=f"xn2{t}", tag=f"xn2{t}") for t in range(2)]
                with tc.tile_pool(name="op", bufs=1) as opool:
                    o_all = [opool.tile([128, SH], F32R, name=f"oal{t}", tag=f"oal{t}")
                             for t in range(2)]

                    # ===================== attention =====================
                    with (
                        tc.tile_pool(name="sc_ps", bufs=3, space="PSUM") as scp,
                        tc.tile_pool(name="ot_ps", bufs=2, space="PSUM") as otp,
                        tc.tile_pool(name="expw", bufs=8) as ep,
                        tc.tile_pool(name="dnw", bufs=2) as dp,
                    ):
                        # software pipeline runs ACROSS (h, qc) chunk
                        # boundaries; exp work is split per k-tile PAIR
                        # between ACT (true exp) and DVE (fp8 bit-trick),
                        # and attn@v contracts each pair in one fp8
                        # DoubleRow matmul.
                        def emit_pair(t, r0, qsl, pi):
                            sc = scp.tile([128, 2 * CK], F32,
                                          name="sc", tag="sc")
                            for j in range(2):
                                kt = 2 * pi + j
                                _mm(nc, sc[:, j * CK:(j + 1) * CK],
                                    kT[t][r0:r0 + 64,
                                          kt * 128:(kt + 1) * 128],
                                    qT[t][r0:r0 + 64, qsl], SCORES_DT,
                                    start=True, stop=True)
                            ex = ep.tile([128, 2 * CK], FP8,
                                         name="ex", tag="ex")
                            if EXP_PAT[pi] == "A":
                                nc.scalar.activation(
                                    ex[:], sc[:], AF.Exp,
                                    bias=expb[:, 0:1], scale=1.0 / SSCALE)
                            else:
                                nc.vector.tensor_scalar(
                                    ex[:].bitcast(I8), sc[:], B0, 0.0,
                                    op0=OP.add, op1=OP.max)
                            return ex

                        def emit_ot(h, ot, pi, ex):
                            t, hh = h // 2, h % 2
                            base = hh * (NP * 2 * VB) + pi * (2 * VB)
                            lhsT = v8[t][:, base:base + 2 * VB].rearrange(
                                "p (j e) -> p j e", j=2)
                            rhs = ex[:].rearrange("p (j q) -> p j q", j=2)
                            nc.tensor.matmul(
                                ot[:], lhsT, rhs,
                                start=(pi == 0), stop=(pi == NP - 1),
                                perf_mode=mybir.MatmulPerfMode.DoubleRow)

                        def emit_norm(t, r0, qsl, ot):
                            # normalize by the denominator (ot row 64):
                            # reciprocal in place on partition 64, DMA the
                            # row down to partition 0, then broadcast.
                            dnr = dp.tile([65, CK], F32, name="dnr", tag="dnr")
                            nc.vector.reciprocal(dnr[64:65, :], ot[64:65, :])
                            rcp = dp.tile([1, CK], F32, name="rcp", tag="rcp")
                            nc.sync.dma_start(rcp[:], dnr[64:65, :])
                            rb = dp.tile([128, CK], F32, name="rb", tag="rb")
                            nc.gpsimd.partition_broadcast(rb[:], rcp[:])
                            if r0 == 0:
                                nc.vector.tensor_mul(
                                    o_all[t][0:64, qsl], ot[0:64, :],
                                    rb[0:64, :])
                            else:
                                om = dp.tile([64, CK], F32R, name="om",
                                             tag="om")
                                nc.vector.tensor_mul(om[:], ot[0:64, :],
                                                     rb[0:64, :])
                                nc.sync.dma_start(o_all[t][64:128, qsl], om[:])

                        LAG = 3
                        chunks = [(h, h // 2, (h % 2) * 64,
                                   slice(qc * CK, (qc + 1) * CK))
                                  for h in range(4 if PHASE >= 2 else 0)
                                  for qc in range(SH // CK)]
                        # one flat software pipeline over all (chunk, pair)
                        # items: each ot matmul trails its score-pair by LAG
                        # pairs uniformly, so chunk boundaries never bunch
                        # the PE stream (a bunched ot drain stalls on exp
                        # latency and starves ACT/DVE at every boundary).
                        ots = {}
                        pend = []

                        def drain_one():
                            ci, pj, exj = pend.pop(0)
                            h, t, r0, qsl = chunks[ci]
                            emit_ot(h, ots[ci], pj, exj)
                            if pj == NP - 1:
                                emit_norm(t, r0, qsl, ots[ci])
                                del ots[ci]

                        for ci, (h, t, r0, qsl) in enumerate(chunks):
                            for pi in range(NP):
                                if pi == 0:
                                    ots[ci] = otp.tile([VB, CK], F32,
                                                       name="ot", tag="ot")
                                pend.append((ci, pi,
                                             emit_pair(t, r0, qsl, pi)))
                                if len(pend) > LAG:
                                    drain_one()
                        while pend:
                            drain_one()

                    if debug:
                        for t in range(2):
                            nc.sync.dma_start(
                                dbg["oall"].ap()[t * 128:(t + 1) * 128, :],
                                o_all[t][:])

                    if PHASE == 2:
                        for t in range(2):
                            nc.sync.dma_start(
                                out_d.ap()[t * 128:(t + 1) * 128, :],
                                o_all[t][:].bitcast(F32))
                    # ============ wo projection + residual 1 =============
                    with tc.tile_pool(name="po_ps", bufs=2, space="PSUM") as pops:
                        # qc outer: both av halves of a chunk finish together,
                        # so LN2 stats become ready in chunk order and the
                        # ACT queue doesn't interleave Ln/Exp with Gelu
                        # (each alternation costs a ~1.3us table reload).
                        for qc in range(SH // CK if PHASE >= 3 else 0):
                            for co in range(2):
                                qsl = slice(qc * CK, (qc + 1) * CK)
                                ap_ = pops.tile([128, CK], F32, name="aops", tag="aops")
                                for ci in range(2):
                                    _mm(nc, ap_[:],
                                        wo_sb[ci][:, co * 128:(co + 1) * 128],
                                        o_all[ci][:, qsl], PROJ_DT,
                                        start=(ci == 0), stop=(ci == 1))
                                nc.vector.scalar_tensor_tensor(
                                    av[co][:, qsl], ap_[:],
                                    vsb["bo_tot"][:, co:co + 1],
                                    x_q[co][:, qsl], op0=OP.add, op1=OP.add)

                if debug:
                    for t in range(2):
                        nc.sync.dma_start(
                            dbg["av"].ap()[t * 128:(t + 1) * 128, :], av[t][:])

                if PHASE == 3:
                    for t in range(2):
                        nc.sync.dma_start(
                            out_d.ap()[t * 128:(t + 1) * 128, :], av[t][:])

                # ==================== LN2 + FFN ==========================
                with (
                    tc.tile_pool(name="ln2w", bufs=2) as lw2,
                    tc.tile_pool(name="ln2_ps", bufs=2, space="PSUM") as lps2,
                    tc.tile_pool(name="ff_ps", bufs=2, space="PSUM") as fps,
                    tc.tile_pool(name="ffw", bufs=2) as fw,
                ):
                    stats2 = []
                    for ch in range(SH // CK if PHASE >= 4 else 0):
                        sl = slice(ch * CK, (ch + 1) * CK)
                        stats2.append(emit_ln_stats(
                            lw2, lps2, av[0][:, sl], av[1][:, sl],
                            x2_act=True, ab_bufs=4, s_dt=F32R))
                    oup = [None, None]
                    for ch in range(SH // CK if PHASE >= 4 else 0):
                        sl = slice(ch * CK, (ch + 1) * CK)
                        Scp2, A2 = stats2[ch]
                        emit_ln_apply(lw2, av[0][:, sl], av[1][:, sl],
                                      Scp2, A2, xn2[0][:, sl], xn2[1][:, sl])
                        g1 = [fw.tile([128, CK], F32R, name=f"g1{fo}", tag=f"g1{fo}")
                              for fo in range(2)]
                        for fo in range(2):
                            f1 = fps.tile([128, CK], F32, name="f1", tag="f1")
                            for ci in range(2):
                                _mm(nc, f1[:],
                                    w1_sb[ci][:, fo * 128:(fo + 1) * 128],
                                    xn2[ci][:, sl], PROJ_DT,
                                    start=(ci == 0), stop=(ci == 1))
                            nc.scalar.activation(g1[fo][:], f1[:], AF.Gelu,
                                                 bias=vsb["b1_tot"][:, fo:fo + 1])
                        for co in range(2):
                            f2 = fps.tile([128, CK], F32, name="f2", tag="f2")
                            for fi in range(2):
                                _mm(nc, f2[:],
                                    w2_sb[fi][:, co * 128:(co + 1) * 128],
                                    g1[fi][:], PROJ_DT,
                                    start=(fi == 0), stop=(fi == 1))
                            # pair output chunks into one tile per co so the
                            # final writeback needs half as many DMA issues
                            if ch % 2 == 0:
                                oup[co] = fw.tile([128, 2 * CK], F32,
                                                  name=f"ou{co}",
                                                  tag=f"ou{co}", bufs=2)
                            ou = oup[co][:, (ch % 2) * CK:(ch % 2 + 1) * CK]
                            nc.vector.scalar_tensor_tensor(
                                ou, f2[:], vsb["b2"][:, co:co + 1],
                                av[co][:, sl].bitcast(F32),
                                op0=OP.add, op1=OP.add)
                            if ch % 2 == 1:
                                psl2 = slice((ch - 1) * CK, (ch + 1) * CK)
                                nc.sync.dma_start(
                                    out_d.ap()[co * 128:(co + 1) * 128, psl2],
                                    oup[co][:])
    nc.finalize()
    return nc


_built = {}


def _get_nc(debug=False):
    key = bool(debug)
    if key not in _built:
        _built[key] = build(debug=debug)
    return _built[key]


def make_in_maps(inputs):
    """Full inputs -> per-core input dicts (core i: batch i//2, half i%2).

    LayerNorm gammas are folded into the projection weights (rows scaled);
    betas are folded into the projection biases; the whole k-bias (bk plus
    wk^T ln1_b) is dropped -- it only adds a per-q-column constant to the
    scores, which softmax ignores.  The score scale SSCALE/8 is folded into
    wq/bq.  bv (plus wv^T ln1_b) is pushed through wo into bo_tot."""
    x = np.ascontiguousarray(np.asarray(inputs["x"], dtype=np.float32))
    x = x.reshape(BS, EMB, SEQ)
    f = lambda k: np.asarray(inputs[k], np.float32)
    wq, wk, wv = f("wq"), f("wk"), f("wv")
    wo, w1, w2 = f("wo"), f("w1"), f("w2")
    bq, bv, bo = f("bq").reshape(HD), f("bv").reshape(HD), f("bo").reshape(EMB)
    b1, b2 = f("b1").reshape(EMB), f("b2").reshape(EMB)
    g1v, b1v = f("ln1_g").reshape(EMB), f("ln1_b").reshape(EMB)
    g2v, b2v = f("ln2_g").reshape(EMB), f("ln2_b").reshape(EMB)
    QS = SSCALE / 8.0
    wqkv = np.zeros((128, 6 * 128), np.float32)
    for wi, (w, sc) in enumerate([(wq, QS), (wk, 1.0), (wv, 1.0)]):
        for t in range(2):
            blk = np.zeros((128, 128), np.float32)
            for hh in range(2):
                gseg = g1v[t * 128 + hh * 64:t * 128 + (hh + 1) * 64]
                blk[hh * 64:(hh + 1) * 64, hh * 64:(hh + 1) * 64] = \
                    w * gseg[:, None] * sc
            wqkv[:, (wi * 2 + t) * 128:(wi * 2 + t + 1) * 128] = blk
    w1g = w1 * g2v[:, None]
    wpk = np.zeros((128, 6 * EMB), np.float32)
    for j, w in enumerate([wo, w1g, w2]):
        wpk[:, (2 * j) * EMB:(2 * j + 1) * EMB] = w[0:128, :]
        wpk[:, (2 * j + 1) * EMB:(2 * j + 2) * EMB] = w[128:256, :]
    bq_tot = np.zeros((2, 128), np.float32)
    bv_tot = np.zeros(EMB, np.float32)
    for h in range(4):
        bseg = b1v[h * 64:(h + 1) * 64]
        bq_tot[h // 2, (h % 2) * 64:(h % 2 + 1) * 64] = \
            (bq + wq.T @ bseg) * QS
        bv_tot[h * 64:(h + 1) * 64] = bv + wv.T @ bseg
    bo_tot = bo + wo.T @ bv_tot
    b1_tot = b1 + w1.T @ b2v
    vecs = np.zeros((128, 8), np.float32)
    for vi, v in enumerate([bo_tot, b1_tot, b2]):
        vecs[:, 2 * vi] = v[0:128]
        vecs[:, 2 * vi + 1] = v[128:256]
    vecs[:, 6] = bq_tot[0]
    vecs[:, 7] = bq_tot[1]
    shared = {
        "wqkv": np.ascontiguousarray(wqkv),
        "wpk": np.ascontiguousarray(wpk),
        "vecs": np.ascontiguousarray(vecs),
    }
    in_maps = []
    for core in range(8):
        b, half = core // 2, core % 2
        xb = x[b]
        if half:
            xb = np.concatenate([xb[:, SH:], xb[:, :SH]], axis=1)
        in_maps.append({"x": np.ascontiguousarray(xb), **shared})
    return in_maps


def assemble(results):
    out = np.empty((BS, EMB, SEQ), np.float32)
    for core in range(8):
        b, half = core // 2, core % 2
        out[b][:, half * SH:(half + 1) * SH] = results[core]["out"]
    return out.reshape(BS, EMB, SZ, SZ)


def kernel(**inputs):
    nc = _get_nc()
    res = bass_utils.run_bass_kernel_spmd(nc, make_in_maps(inputs),
                                          core_ids=list(range(8)))
    return assemble(res.results)

